# revision 3
# baseline (speedup 1.0000x reference)
"""Trainium2 Bass kernel for nn_AttentionBlock (causal bilinear attention).

Reference computation (N_NET=1, D=4, N_H=8, N_T=2048, N_IN=N_OUT=256):
    Omega[N,b,h,t,u] = r'[N,b,t,i] Q[N,h,i,j] r'[N,b,u,j]
    Omega *= tril(ones(T, T))                      # causal mask
    r[N,b,t,i] = Omega[N,b,h,t,u] E[N,h,i,j] r'[N,b,u,j]   # sums over h

This is causal LINEAR attention (no softmax), so instead of materializing
the 2048x2048 Omega triangle (the baseline approach, ~344k PE columns per
core) we use the chunked state-passing formulation (~164k PE columns):

    P_h = r' Q_h            (queries)       V_h = r' E_h^T   (values)
    out[t] = sum_h [ sum_{u in chunk(t), u<=t} (P_h[t].r'[u]) V_h[u]   (intra)
                     + P_h[t] . S_h(chunk(t)) ]                        (inter)
    S_h(n) = sum_{u < 128n} r'[u]^T V_h[u]   (256x256 running state,
                                              accumulated in PSUM)

Sharding across 8 NeuronCores: core c handles batch b = c//2 and the 4
heads [4*(c%2), 4*(c%2)+4); the host adds the two head-group partials.

PSUM accumulation constraint (measured on HW): each bank honors start=False
accumulation only for the address window opened by the most recent
start=True matmul on that bank.  All accumulation groups here therefore
write one contiguous window per bank: head-PAIRS are packed into a single
moving operand (512 cols) for the V/state matmuls and into pair-interleaved
PT tiles for the Omega matmuls.

Per-core layouts (K = contraction on partitions; M = psum partitions):
    PTp[p][jc](j, n, h2, t) = sum_i Q(i,j) rT(i,t)       (A phase, P^T)
    V[n][p](u, h2, i)       = sum_j rT(j,u_n) ET(j,i)    (V phase)
    OmT_p(u, h2, t)         = sum_j rT(j,u_n) PTp(j,n,h2,t), masked u<=t
    S_ps[p][jb](j, h2, i)  += sum_u rn(u_n,j) V[n][p]    (PSUM state accum)
    po(t,i)                 = sum_u OmT(u,t) V + sum_j PTp(j,n,h2,t) S_sb

Output is produced directly in natural (t, i) layout -> no host transpose.
float16 matmuls (full PE rate, fast weight loads); PSUM accumulates fp32.
"""

import numpy as np

N_T = 2048
N_IN = 256
C = 128             # chunk size
NCH = N_T // C      # 16 chunks
T_TILE = 512
TT = N_T // T_TILE  # 4 t-tiles in A phase
HL = 4              # heads per core
N_CORES = 8

_cache = {}


def _tri_mask():
    # mask[p=u, c=t] = 1 if t >= u  (keep u <= t on the diagonal block)
    idx = np.arange(128)
    return (idx[None, :] >= idx[:, None]).astype(np.float32)


def _build_nc(repeat=1, bf16="fp16"):
    import concourse.tile as tile
    import concourse.mybir as mybir
    from concourse import bacc

    F32 = mybir.dt.float32
    FMM = {False: mybir.dt.float32r, True: mybir.dt.bfloat16,
           "fp16": mybir.dt.float16}[bf16]

    nc = bacc.Bacc("TRN2", target_bir_lowering=False, debug=False,
                   num_devices=N_CORES)
    rT_d = nc.dram_tensor("rT", (2, 128, N_T), FMM, kind="ExternalInput").ap()
    rn_d = nc.dram_tensor("rn", (NCH, 128, N_IN), FMM,
                          kind="ExternalInput").ap()
    Q4_d = nc.dram_tensor("Q4", (HL, 2, 128, N_IN), FMM,
                          kind="ExternalInput").ap()
    ET4_d = nc.dram_tensor("ET4", (HL, 2, 128, N_IN), FMM,
                           kind="ExternalInput").ap()
    mask_d = nc.dram_tensor("mask", (128, 128), FMM,
                            kind="ExternalInput").ap()
    out_d = nc.dram_tensor("out", (NCH, 128, N_IN), F32,
                           kind="ExternalOutput").ap()

    # running per-engine copy-cost estimates for greedy DVE/ACT balancing
    eng_load = {"v": 0.0, "s": 0.0}

    def copy_psum(out_ap, in_ap, n):
        dve = n / 0.96 + 150.0
        act = (n + 352.0) / 1.2
        if eng_load["v"] + dve <= eng_load["s"] + act:
            eng_load["v"] += dve
            nc.vector.tensor_copy(out_ap, in_ap)
        else:
            eng_load["s"] += act
            nc.scalar.copy(out_ap, in_ap)

    with tile.TileContext(nc) as tc:
        with (
            tc.tile_pool(name="const", bufs=1) as const,
            tc.tile_pool(name="om_pool", bufs=8) as om_pool,
            tc.tile_pool(name="opool", bufs=4) as opool,
            tc.tile_pool(name="psS", bufs=1, space="PSUM") as psS,
            tc.tile_pool(name="pswork", bufs=2, space="PSUM") as pswork,
            tc.tile_pool(name="psout", bufs=2, space="PSUM") as psout,
        ):
            # --- PE warm-up: dummy matmuls on memset data run during the
            # input-DMA lead-in so the HAM un-throttles (1.2->2.4 GHz)
            # before the first real matmul ---
            warm_f32 = const.tile([128, 128], F32)
            nc.vector.memset(warm_f32, 0.0)
            warm_sb = const.tile([128, 128], FMM)
            nc.vector.tensor_copy(warm_sb, warm_f32)
            warm_ps = pswork.tile([128, T_TILE], F32, tag="w", name="warm_ps")
            for _w in range(24):
                nc.tensor.matmul(warm_ps[:, :128], warm_sb, warm_sb,
                                 start=True, stop=True, skip_group_check=True)

            # --- input tiles; DMA order matches consumption order ---
            mask_sb = const.tile([128, 128], FMM)
            Q_h = [const.tile([128, 2, N_IN], FMM, name=f"Qh{h}")
                   for h in range(HL)]
            rT_sb = [const.tile([128, N_T], FMM, name=f"rT{jc}")
                     for jc in range(2)]
            rn_sb = const.tile([128, NCH, N_IN], FMM, name="rn_sb")
            ET_p = [const.tile([128, 2, 2, N_IN], FMM, name=f"ETp{p}")
                    for p in range(2)]
            for h in range(HL):
                for ic in range(2):
                    nc.sync.dma_start(out=Q_h[h][:, ic, :], in_=Q4_d[h, ic])
            for tq in range(TT):
                for ic in range(2):
                    nc.sync.dma_start(
                        out=rT_sb[ic][:, T_TILE * tq:T_TILE * (tq + 1)],
                        in_=rT_d[ic, :, T_TILE * tq:T_TILE * (tq + 1)])
            for p in range(2):
                for jc in range(2):
                    for h2 in range(2):
                        nc.sync.dma_start(out=ET_p[p][:, jc, h2, :],
                                          in_=ET4_d[2 * p + h2, jc])
            nc.sync.dma_start(out=mask_sb, in_=mask_d)
            for n in range(NCH):
                nc.sync.dma_start(out=rn_sb[:, n, :], in_=rn_d[n])

            # P^T, pair-interleaved chunk-major: [j, chunk, h2, t]
            PTp = [[const.tile([128, NCH, 2, C], FMM, name=f"PTp{p}_{jc}")
                    for jc in range(2)] for p in range(2)]
            V_t = [[const.tile([128, 2, N_IN], FMM, name=f"V{n}_{p}")
                    for p in range(2)] for n in range(NCH)]
            # state accumulators: bank per (pair, j-block): [j, h2, i]
            S_ps = [[psS.tile([128, 2, N_IN], F32, name=f"Sps{p}_{jb}")
                     for jb in range(2)] for p in range(2)]
            S_sb = [[[const.tile([128, 2, N_IN], FMM, name=f"Ssb{p}_{jb}_{b}")
                      for b in range(2)] for jb in range(2)] for p in range(2)]

            def body():
                # ---- Phase A: P^T = Q^T rT, tq-major for DMA overlap
                for tq in range(TT):
                    ts = slice(T_TILE * tq, T_TILE * (tq + 1))
                    for p in range(2):
                        for h2 in range(2):
                            h = 2 * p + h2
                            for jc in range(2):
                                ps = pswork.tile([128, T_TILE], F32, tag="w",
                                                 name="ps_a")
                                for ic in range(2):
                                    nc.tensor.matmul(
                                        ps,
                                        Q_h[h][:, ic, 128 * jc:128 * (jc + 1)],
                                        rT_sb[ic][:, ts],
                                        start=(ic == 0), stop=(ic == 1))
                                # strided copy into 4 chunk slots
                                copy_psum(
                                    PTp[p][jc][:, 4 * tq:4 * tq + 4, h2, :],
                                    ps, T_TILE)
                # ---- Phase V: V[n][pair] = rT_chunk^T ET_pair
                for n in range(NCH):
                    cs = slice(C * n, C * (n + 1))
                    pv = [pswork.tile([128, 2, N_IN], F32, tag="w",
                                      name=f"ps_v{p}") for p in range(2)]
                    for jc in range(2):
                        for p in range(2):
                            nc.tensor.matmul(pv[p], rT_sb[jc][:, cs],
                                             ET_p[p][:, jc, :, :],
                                             start=(jc == 0), stop=(jc == 1),
                                             skip_group_check=True)
                    for p in range(2):
                        copy_psum(V_t[n][p], pv[p], 2 * N_IN)
                # ---- Main loop over chunks ----
                for n in range(NCH):
                    cs = slice(C * n, C * (n + 1))
                    # OmT[u, h2, t] per pair; one 256-col window per bank
                    w = [pswork.tile([128, 2, C], F32, tag="w",
                                     name=f"ps_om{p}") for p in range(2)]
                    for jc in range(2):
                        for p in range(2):
                            nc.tensor.matmul(w[p], rT_sb[jc][:, cs],
                                             PTp[p][jc][:, n, :, :],
                                             start=(jc == 0), stop=(jc == 1),
                                             skip_group_check=True)
                    om = []
                    for h in range(HL):
                        o = om_pool.tile([128, C], FMM, tag="om", name="om")
                        nc.vector.tensor_mul(o, w[h // 2][:, h % 2, :],
                                             mask_sb)
                        eng_load["v"] += 128 / 0.96 + 150.0
                        om.append(o)
                    # output accumulation for this chunk: inter + intra
                    po = psout.tile([128, N_IN], F32, tag="po", name="po")
                    n_mm = HL * (3 if n > 0 else 1)
                    k = 0
                    if n > 0:
                        for p in range(2):
                            for h2 in range(2):
                                for jc in range(2):
                                    nc.tensor.matmul(
                                        po, PTp[p][jc][:, n, h2, :],
                                        S_sb[p][jc][(n - 1) % 2][:, h2, :],
                                        start=(k == 0), stop=(k == n_mm - 1),
                                        skip_group_check=True)
                                    k += 1
                    for h in range(HL):
                        nc.tensor.matmul(po, om[h], V_t[n][h // 2][:, h % 2, :],
                                         start=(k == 0), stop=(k == n_mm - 1),
                                         skip_group_check=True)
                        k += 1
                    # state update + tap to SBUF (last chunk's state unused);
                    # full-bank window per (pair, jb) -> one matmul each
                    if n < NCH - 1:
                        for jb in range(2):
                            for p in range(2):
                                nc.tensor.matmul(
                                    S_ps[p][jb],
                                    rn_sb[:, n, 128 * jb:128 * (jb + 1)],
                                    V_t[n][p][:, :, :],
                                    start=(n == 0), stop=True,
                                    skip_group_check=True)
                        for p in range(2):
                            for jb in range(2):
                                copy_psum(S_sb[p][jb][n % 2], S_ps[p][jb],
                                          2 * N_IN)
                    # drain chunk output
                    ot = opool.tile([128, N_IN], F32, tag="ot", name="ot")
                    copy_psum(ot, po, N_IN)
                    nc.sync.dma_start(out=out_d[n], in_=ot)

            if repeat == 1:
                body()
            elif repeat < 0:  # unrolled repeat (timing experiments)
                for _ in range(-repeat):
                    body()
            else:
                with tc.For_i(0, repeat, 1):
                    body()
    nc.compile()
    return nc


def _prep_in_maps(r_prime, E, Q, bf16="fp16"):
    if bf16 == "fp16":
        cast_dt = np.float16
    elif bf16:
        import ml_dtypes
        cast_dt = ml_dtypes.bfloat16
    else:
        cast_dt = np.float32
    mask = _tri_mask()
    in_maps = []
    for c in range(N_CORES):
        b, hg = divmod(c, 2)
        heads = slice(4 * hg, 4 * hg + 4)
        rT = np.ascontiguousarray(r_prime[0, b].T).reshape(2, 128, N_T)
        rn = np.ascontiguousarray(r_prime[0, b]).reshape(NCH, 128, N_IN)
        Q4 = np.ascontiguousarray(Q[0, heads]).reshape(HL, 2, 128, N_IN)
        ET4 = np.ascontiguousarray(
            E[0, heads].transpose(0, 2, 1)).reshape(HL, 2, 128, N_IN)
        in_maps.append({"rT": rT.astype(cast_dt),
                        "rn": rn.astype(cast_dt),
                        "Q4": Q4.astype(cast_dt),
                        "ET4": ET4.astype(cast_dt),
                        "mask": mask.astype(cast_dt)})
    return in_maps


DTYPE = "fp16"  # float16 matmuls: full PE rate + fast weight loads


def kernel(r_prime, E, Q):
    from concourse import bass_utils

    if "nc" not in _cache:
        _cache["nc"] = _build_nc(bf16=DTYPE)
    nc = _cache["nc"]
    in_maps = _prep_in_maps(r_prime, E, Q, bf16=DTYPE)
    res = bass_utils.run_bass_kernel_spmd(nc, in_maps,
                                          core_ids=list(range(N_CORES)))
    out = np.zeros((1, 4, N_T, N_IN), dtype=np.float32)
    for b in range(4):
        out[0, b] = (res.results[2 * b]["out"]
                     + res.results[2 * b + 1]["out"]).reshape(N_T, N_IN)
    return out
